# revision 6
# baseline (speedup 1.0000x reference)
"""Trainium2 Bass kernel: single-head causal self-attention.

Problem: x[B=8, S=2048, D=1024], Wq/Wk/Wv[D, H=64], bq/bk/bv[H].
    q = x@Wq+bq; k = x@Wk+bk; v = x@Wv+bv
    out = softmax(causal(q k^T) / sqrt(H)) @ v

Sharding: batch -> 8 NeuronCores (data parallel, no collectives).

Per-core layout strategy (all fp32 storage, fp32r matmuls):
  - host pre-transposes the x shard to xT [D, S] so all projection
    matmuls contract over D on the partition axis with contiguous DMA
  - Wq|Wk packed into one [D, 128] stationary operand -> Q^T and K^T
    land stacked in one [128, S] SBUF tile (full PE width)
  - Wv gets an appended ones column (bias 1.0) so P @ [V|1] yields the
    softmax denominator as an extra output column (no separate rowsum)
  - scores are built transposed: S^T tile [128 k, 512 q] = K_i Q^T, exp
    on the scalar engine (scale=1/8) gives P^T tiles directly in SBUF,
    which are the moving operand of O^T = (V|1)^T P^T
  - causal mask: multiplicative 0/1 masks on the 4 diagonal-block
    shapes, applied after exp (exact); off-diagonal k>q tiles skipped
  - O^T [65, S] is PE-transposed back in [65,128] chunks, each chunk is
    normalized by the reciprocal of its ones-column and DMA'd out
"""

import sys

sys.path.insert(0, "/opt/trn_rl_repo")

import numpy as np

B, S, D, H = 8, 2048, 1024, 64
N_CORES = 8
SQ = 512            # q free-chunk (fp32 moving max / one PSUM bank)
SK = 128            # k chunk (PE contraction width for the O matmul)
NQ = S // SQ        # 4
NK = S // SK        # 16
ND = D // 128       # 8 contraction chunks for projections
H1 = H + 1          # V plus ones column

_CACHE = {}


def _build_nc():
    import concourse.tile as tile
    from concourse import bacc, mybir

    f32 = mybir.dt.float32
    f32r = mybir.dt.float32r
    AF = mybir.ActivationFunctionType

    nc = bacc.Bacc(None, target_bir_lowering=False)
    xT = nc.dram_tensor("xT", [D, S], f32r, kind="ExternalInput")
    wqk = nc.dram_tensor("wqk", [D, 2 * H], f32r, kind="ExternalInput")
    wv1 = nc.dram_tensor("wv1", [D, H1], f32r, kind="ExternalInput")
    bqk = nc.dram_tensor("bqk", [2 * H, 1], f32, kind="ExternalInput")
    bv1 = nc.dram_tensor("bv1", [H1, 1], f32, kind="ExternalInput")
    masks = nc.dram_tensor("masks", [128, 4 * SQ], f32r, kind="ExternalInput")
    ident = nc.dram_tensor("ident", [128, 128], f32r, kind="ExternalInput")
    out = nc.dram_tensor("out", [S, H], f32, kind="ExternalOutput")

    def r(ap):
        return ap

    with tile.TileContext(nc) as tc:
        from contextlib import ExitStack

        with ExitStack() as ctx:
            const = ctx.enter_context(tc.tile_pool(name="const", bufs=1))
            sb = ctx.enter_context(tc.tile_pool(name="sb", bufs=1))
            pt_pool = ctx.enter_context(tc.tile_pool(name="pt", bufs=6))
            o_pool = ctx.enter_context(tc.tile_pool(name="o", bufs=4))
            ps_big = ctx.enter_context(
                tc.tile_pool(name="psb", bufs=4, space="PSUM")
            )
            ps_sm = ctx.enter_context(
                tc.tile_pool(name="pss", bufs=2, space="PSUM")
            )
            ps_ot = ctx.enter_context(
                tc.tile_pool(name="pso", bufs=2, space="PSUM")
            )

            # ---- loads ----
            xT_sb = sb.tile([128, ND * S], f32r)
            for c in range(ND):
                nc.sync.dma_start(
                    xT_sb[:, c * S : (c + 1) * S], xT[c * 128 : (c + 1) * 128, :]
                )
            wqk_sb = const.tile([128, ND * 2 * H], f32r)
            nc.sync.dma_start(
                wqk_sb[:].rearrange("p (c m) -> p c m", c=ND),
                wqk.rearrange("(c p) m -> p c m", p=128),
            )
            wv1_sb = const.tile([128, ND * H1], f32r)
            nc.sync.dma_start(
                wv1_sb[:].rearrange("p (c m) -> p c m", c=ND),
                wv1.rearrange("(c p) m -> p c m", p=128),
            )
            bqk_sb = const.tile([128, 1], f32)
            nc.sync.dma_start(bqk_sb[:], bqk[:, :])
            bv1_sb = const.tile([H1, 1], f32)
            nc.sync.dma_start(bv1_sb[:], bv1[:, :])
            masks_sb = const.tile([128, 4 * SQ], f32r)
            nc.sync.dma_start(masks_sb[:], masks[:, :])
            ident_sb = const.tile([128, 128], f32r)
            nc.sync.dma_start(ident_sb[:], ident[:, :])

            # ---- projections: QK^T stacked [128, S]; V~^T [65, S] ----
            QKT_sb = sb.tile([128, S], f32r)
            for j in range(NQ):
                ps = ps_big.tile([128, SQ], f32, tag="psb")
                for c in range(ND):
                    nc.tensor.matmul(
                        ps[:],
                        r(wqk_sb[:, c * 2 * H : (c + 1) * 2 * H]),
                        r(xT_sb[:, c * S + j * SQ : c * S + j * SQ + SQ]),
                        start=(c == 0),
                        stop=(c == ND - 1),
                    )
                nc.scalar.activation(
                    QKT_sb[:, j * SQ : (j + 1) * SQ], ps[:], AF.Identity,
                    bias=bqk_sb[:],
                )
            # K^T half must be re-based to partition 0 for the S^T matmuls
            # (matmul operands must share base_partition) — SBUF->SBUF DMA
            KT0_sb = sb.tile([H, S], f32r)
            nc.sync.dma_start(KT0_sb[:], QKT_sb[H : 2 * H, :])
            VT1_sb = sb.tile([H1, S], f32r)
            for j in range(NQ):
                ps = ps_big.tile([H1, SQ], f32, tag="psb")
                for c in range(ND):
                    nc.tensor.matmul(
                        ps[:],
                        r(wv1_sb[:, c * H1 : (c + 1) * H1]),
                        r(xT_sb[:, c * S + j * SQ : c * S + j * SQ + SQ]),
                        start=(c == 0),
                        stop=(c == ND - 1),
                    )
                nc.scalar.activation(
                    VT1_sb[:, j * SQ : (j + 1) * SQ], ps[:], AF.Identity,
                    bias=bv1_sb[:],
                )

            # ---- V~ = (V|1) in [s, h'] layout via PE transposes ----
            Vones_sb = sb.tile([128, NK * H1], f32r)
            for t in range(NK):
                pst = ps_sm.tile([128, H1], f32, tag="pss")
                nc.tensor.transpose(
                    pst[:],
                    VT1_sb[:, t * 128 : (t + 1) * 128].bitcast(f32),
                    ident_sb[:H1, :H1].bitcast(f32),
                )
                nc.vector.tensor_copy(Vones_sb[:, t * H1 : (t + 1) * H1], pst[:])

            # ---- attention: S^T -> exp -> mask -> O^T accumulate ----
            OT_sb = sb.tile([H1, S], f32)
            for J in range(NQ):
                ot = ps_ot.tile([H1, SQ], f32, tag="pso")
                nk = 4 * (J + 1)
                for i in range(nk):
                    st = ps_big.tile([128, SQ], f32, tag="psb")
                    nc.tensor.matmul(
                        st[:],
                        r(KT0_sb[:, i * 128 : (i + 1) * 128]),
                        r(QKT_sb[:H, J * SQ : (J + 1) * SQ]),
                        start=True,
                        stop=True,
                    )
                    pt = pt_pool.tile([128, SQ], f32r, tag="pt")
                    nc.scalar.activation(pt[:], st[:], AF.Exp, scale=0.125)
                    rr = i - 4 * J
                    if rr >= 0:
                        nc.vector.tensor_mul(
                            pt[:], pt[:], masks_sb[:, rr * SQ : (rr + 1) * SQ]
                        )
                    nc.tensor.matmul(
                        ot[:],
                        r(Vones_sb[:, i * H1 : (i + 1) * H1]),
                        r(pt[:]),
                        start=(i == 0),
                        stop=(i == nk - 1),
                    )
                nc.vector.tensor_copy(OT_sb[:, J * SQ : (J + 1) * SQ], ot[:])

            # ---- normalize + output ----
            for t in range(NK):
                po = ps_sm.tile([128, H1], f32, tag="pss")
                nc.tensor.transpose(
                    po[:],
                    OT_sb[:, t * 128 : (t + 1) * 128],
                    ident_sb[:H1, :H1].bitcast(f32),
                )
                rc = o_pool.tile([128, 1], f32, tag="rc")
                nc.vector.reciprocal(rc[:], po[:, H : H + 1])
                ob = o_pool.tile([128, H], f32, tag="ob")
                nc.vector.tensor_scalar_mul(ob[:], po[:, :H], rc[:])
                nc.sync.dma_start(out[t * 128 : (t + 1) * 128, :], ob[:])

    nc.finalize()
    return nc


def _host_prep(x, Wq, bq, Wk, bk, Wv, bv):
    """Layout-only host prep: shard x by batch + pack weight operands."""
    f32 = np.float32
    wqk = np.ascontiguousarray(
        np.concatenate([Wq, Wk], axis=1), dtype=f32
    )  # [D, 128]
    wv1 = np.ascontiguousarray(
        np.concatenate([Wv, np.zeros((D, 1), f32)], axis=1), dtype=f32
    )  # [D, 65]
    bqk = np.ascontiguousarray(
        np.concatenate([bq, bk])[:, None], dtype=f32
    )  # [128, 1]
    bv1 = np.ascontiguousarray(
        np.concatenate([bv, np.ones((1,), f32)])[:, None], dtype=f32
    )  # [65, 1]
    # diagonal-block causal masks: tile (k-chunk i, q-chunk J), r = i-4J:
    # keep (512J + qq) >= (128i + kk)  <=>  qq >= 128r + kk
    kk = np.arange(128)[:, None]
    qq = np.arange(SQ)[None, :]
    masks = np.concatenate(
        [(qq >= 128 * r + kk).astype(f32) for r in range(4)], axis=1
    )  # [128, 2048]
    ident = np.eye(128, dtype=f32)
    common = {
        "wqk": wqk, "wv1": wv1, "bqk": bqk, "bv1": bv1,
        "masks": masks, "ident": ident,
    }
    in_maps = []
    for b in range(B):
        m = dict(common)
        m["xT"] = np.ascontiguousarray(x[b].T, dtype=f32)  # [D, S]
        in_maps.append(m)
    return in_maps


def run(x, Wq, bq, Wk, bk, Wv, bv, trace=False):
    from concourse.bass_utils import run_bass_kernel_spmd

    if "nc" not in _CACHE:
        _CACHE["nc"] = _build_nc()
    nc = _CACHE["nc"]
    in_maps = _host_prep(
        np.asarray(x), np.asarray(Wq), np.asarray(bq), np.asarray(Wk),
        np.asarray(bk), np.asarray(Wv), np.asarray(bv),
    )
    res = run_bass_kernel_spmd(
        nc, in_maps, core_ids=list(range(N_CORES)), trace=trace
    )
    outs = np.stack([res.results[c]["out"] for c in range(N_CORES)], axis=0)
    return outs.astype(np.float32), res


def kernel(x, Wq, bq, Wk, bk, Wv, bv):
    outs, _ = run(x, Wq, bq, Wk, bk, Wv, bv, trace=False)
    return outs


# revision 7
# speedup vs baseline: 1.1165x; 1.1165x over previous
"""Trainium2 Bass kernel: single-head causal self-attention.

Problem: x[B=8, S=2048, D=1024], Wq/Wk/Wv[D, H=64], bq/bk/bv[H].
    q = x@Wq+bq; k = x@Wk+bk; v = x@Wv+bv
    out = softmax(causal(q k^T) / sqrt(H)) @ v

Sharding: batch -> 8 NeuronCores (data parallel, no collectives).

Per-core strategy:
  - host pre-transposes the x shard to xT [D, S]; projections contract
    over D on the partition axis with contiguous DMA (fp32r matmuls)
  - Wq|Wk packed into one [D, 128] stationary operand (full PE width);
    Wv carries an appended zero column whose bias is 1.0, so P @ [V|1]
    yields the softmax denominator as an extra output column
  - x is loaded in 512-seq blocks; each q-chunk's projections and its
    causal attention rows run as soon as their block lands (pipelined)
  - scores are built transposed: S^T [128 k, 512 q] = K_i Q^T in PSUM,
    exp (scale=1/8, bf16 out) over two PSUM banks at a time gives P^T
    tiles in SBUF = the moving operand of O^T = (V|1)^T P^T (bf16)
  - causal masks: multiplicative 0/1 bf16 masks over the two
    diagonal-block pairs of each q-chunk, applied after exp (exact)
  - O^T [65, 512] chunks are PE-transposed back, normalized by the
    reciprocal of the ones-column, and DMA'd out per q-chunk
"""

import sys

sys.path.insert(0, "/opt/trn_rl_repo")

import numpy as np

B, S, D, H = 8, 2048, 1024, 64
N_CORES = 8
SQ = 512            # q chunk (PSUM bank / fp32 moving max)
NQ = S // SQ        # 4
ND = D // 128       # 8 contraction chunks for projections
H1 = H + 1          # V plus ones column

_CACHE = {}


def _build_nc():
    import concourse.tile as tile
    from concourse import bacc, mybir

    f32 = mybir.dt.float32
    f32r = mybir.dt.float32r
    bf16 = mybir.dt.bfloat16
    AF = mybir.ActivationFunctionType

    nc = bacc.Bacc(None, target_bir_lowering=False)
    xT = nc.dram_tensor("xT", [D, S], f32r, kind="ExternalInput")
    wqk = nc.dram_tensor("wqk", [128, ND * 2 * H], f32r, kind="ExternalInput")
    wv1 = nc.dram_tensor("wv1", [128, ND * H1], f32r, kind="ExternalInput")
    bqk = nc.dram_tensor("bqk", [2 * H, 1], f32, kind="ExternalInput")
    bv1 = nc.dram_tensor("bv1", [H1, 1], f32, kind="ExternalInput")
    masks = nc.dram_tensor("masks", [128, 4 * SQ], bf16, kind="ExternalInput")
    ident = nc.dram_tensor("ident", [128, 128], f32, kind="ExternalInput")
    out = nc.dram_tensor("out", [S, H], f32, kind="ExternalOutput")

    with tile.TileContext(nc) as tc:
        from contextlib import ExitStack

        with ExitStack() as ctx:
            const = ctx.enter_context(tc.tile_pool(name="const", bufs=1))
            sb = ctx.enter_context(tc.tile_pool(name="sb", bufs=1))
            pt_pool = ctx.enter_context(tc.tile_pool(name="pt", bufs=4))
            o_pool = ctx.enter_context(tc.tile_pool(name="o", bufs=2))
            ps = ctx.enter_context(tc.tile_pool(name="ps", bufs=1, space="PSUM"))

            # ---- constant loads (small) ----
            wqk_sb = const.tile([128, ND * 2 * H], f32r)
            nc.sync.dma_start(wqk_sb[:], wqk[:, :])
            wv1_sb = const.tile([128, ND * H1], f32r)
            nc.sync.dma_start(wv1_sb[:], wv1[:, :])
            bqk_sb = const.tile([128, 1], f32)
            nc.sync.dma_start(bqk_sb[:], bqk[:, :])
            bv1_sb = const.tile([H1, 1], f32)
            nc.sync.dma_start(bv1_sb[:], bv1[:, :])
            masks_sb = const.tile([128, 4 * SQ], bf16)
            nc.sync.dma_start(masks_sb[:], masks[:, :])
            ident_sb = const.tile([128, 128], f32)
            nc.sync.dma_start(ident_sb[:], ident[:, :])

            xt = {}          # (c, sj) -> [128, SQ] f32r
            QKT = {}         # J -> [128, SQ] bf16 (Q^T rows 0:64, K^T 64:128)
            KT0 = {}         # J -> [64, SQ] bf16 at base partition 0
            Vones = {}       # t -> [128, H1] bf16 ((V|1) rows for k-chunk t)

            for J in range(NQ):
                # ---- x block J lands (split across sync + gpsimd DGEs) ----
                with nc.named_scope(f"load{J}"):
                    for c in range(ND):
                        t_x = sb.tile([128, SQ], f32r, tag=f"x{c}_{J}")
                        eng = nc.sync if c % 2 == 0 else nc.gpsimd
                        eng.dma_start(
                            t_x[:], xT[c * 128 : (c + 1) * 128,
                                       J * SQ : (J + 1) * SQ]
                        )
                        xt[(c, J)] = t_x

                # ---- projections for q-chunk J ----
                with nc.named_scope(f"proj{J}"):
                    qk = ps.tile([128, SQ], f32, tag="proj", bufs=2)
                    for c in range(ND):
                        nc.tensor.matmul(
                            qk[:],
                            wqk_sb[:, c * 2 * H : (c + 1) * 2 * H],
                            xt[(c, J)][:],
                            start=(c == 0),
                            stop=(c == ND - 1),
                        )
                    qkt = sb.tile([128, SQ], bf16, tag=f"qkt{J}")
                    nc.scalar.activation(
                        qkt[:], qk[:], AF.Identity, bias=bqk_sb[:]
                    )
                    QKT[J] = qkt
                    kt0 = sb.tile([H, SQ], bf16, tag=f"kt0{J}")
                    nc.sync.dma_start(kt0[:], qkt[H : 2 * H, :])
                    KT0[J] = kt0

                    vv = ps.tile([H1, SQ], f32, tag="proj", bufs=2)
                    for c in range(ND):
                        nc.tensor.matmul(
                            vv[:],
                            wv1_sb[:, c * H1 : (c + 1) * H1],
                            xt[(c, J)][:],
                            start=(c == 0),
                            stop=(c == ND - 1),
                        )
                    vt1 = sb.tile([H1, SQ], f32, tag=f"vt1{J}")
                    nc.vector.tensor_scalar_add(vt1[:], vv[:], bv1_sb[:])
                    # V~ = (V|1) in [s, h'] rows via PE transposes
                    for tt in range(4):
                        t_k = 4 * J + tt
                        pst = ps.tile([128, H1], f32, tag="st", bufs=2)
                        nc.tensor.transpose(
                            pst[:],
                            vt1[:, tt * 128 : (tt + 1) * 128],
                            ident_sb[:H1, :H1],
                        )
                        vo = sb.tile([128, H1], bf16, tag=f"vo{t_k}")
                        nc.vector.tensor_copy(vo[:], pst[:])
                        Vones[t_k] = vo

                # ---- causal attention rows for q-chunk J ----
                with nc.named_scope(f"att{J}"):
                    ot = ps.tile([H1, SQ], f32, tag="ot", bufs=2)
                    nhalf = 2 * (J + 1)   # pairs of k-chunks
                    for ii in range(nhalf):
                        st = ps.tile([128, 2 * SQ], f32, tag="st", bufs=2)
                        for h2 in range(2):
                            i = 2 * ii + h2
                            nc.tensor.matmul(
                                st[:, h2 * SQ : (h2 + 1) * SQ],
                                KT0[i // 4][:, (i % 4) * 128 : (i % 4 + 1) * 128],
                                QKT[J][:H, :],
                                start=True,
                                stop=True,
                            )
                        pt = pt_pool.tile([128, 2 * SQ], bf16, tag="pt")
                        nc.scalar.activation(pt[:], st[:], AF.Exp, scale=0.125)
                        if ii >= 2 * J:   # diagonal pair: apply causal mask
                            half = ii - 2 * J   # 0 -> r=0,1 ; 1 -> r=2,3
                            nc.vector.tensor_mul(
                                pt[:],
                                pt[:],
                                masks_sb[:, half * 2 * SQ : (half + 1) * 2 * SQ],
                            )
                        for h2 in range(2):
                            i = 2 * ii + h2
                            nc.tensor.matmul(
                                ot[:],
                                Vones[i][:],
                                pt[:, h2 * SQ : (h2 + 1) * SQ],
                                start=(i == 0),
                                stop=(i == 4 * (J + 1) - 1),
                            )

                # ---- normalize + store rows 512J..512J+511 ----
                with nc.named_scope(f"out{J}"):
                    ots = sb.tile([H1, SQ], f32, tag=f"ots{J}")
                    nc.vector.tensor_copy(ots[:], ot[:])
                    ob = o_pool.tile([128, 4 * H], f32, tag="ob")
                    for tt in range(4):
                        po = ps.tile([128, H1], f32, tag="st", bufs=2)
                        nc.tensor.transpose(
                            po[:],
                            ots[:, tt * 128 : (tt + 1) * 128],
                            ident_sb[:H1, :H1],
                        )
                        rc = o_pool.tile([128, 1], f32, tag="rc")
                        nc.vector.reciprocal(rc[:], po[:, H : H + 1])
                        nc.vector.tensor_scalar_mul(
                            ob[:, tt * H : (tt + 1) * H], po[:, :H], rc[:]
                        )
                    nc.sync.dma_start(
                        out[J * SQ : (J + 1) * SQ, :].rearrange(
                            "(t p) h -> p t h", p=128
                        ),
                        ob[:].rearrange("p (t h) -> p t h", t=4),
                    )

    nc.finalize()
    return nc


def _host_prep(x, Wq, bq, Wk, bk, Wv, bv):
    """Layout-only host prep: shard x by batch + pack weight operands."""
    import ml_dtypes

    f32 = np.float32
    wqk = np.concatenate([Wq, Wk], axis=1)          # [D, 128]
    # pack [D, M] -> [128, ND*M]: chunk c of 128 D-rows at cols c*M..
    wqk = np.ascontiguousarray(
        wqk.reshape(ND, 128, 2 * H).transpose(1, 0, 2).reshape(128, ND * 2 * H),
        dtype=f32,
    )
    wv1 = np.concatenate([Wv, np.zeros((D, 1), f32)], axis=1)  # [D, 65]
    wv1 = np.ascontiguousarray(
        wv1.reshape(ND, 128, H1).transpose(1, 0, 2).reshape(128, ND * H1),
        dtype=f32,
    )
    bqk = np.ascontiguousarray(np.concatenate([bq, bk])[:, None], dtype=f32)
    bv1 = np.ascontiguousarray(
        np.concatenate([bv, np.ones((1,), f32)])[:, None], dtype=f32
    )
    # diagonal-block causal masks: tile (k-chunk i, q-chunk J), r = i-4J:
    # keep (512J + qq) >= (128i + kk)  <=>  qq >= 128r + kk
    kk = np.arange(128)[:, None]
    qq = np.arange(SQ)[None, :]
    masks = np.concatenate(
        [(qq >= 128 * r + kk) for r in range(4)], axis=1
    ).astype(ml_dtypes.bfloat16)  # [128, 2048]
    ident = np.eye(128, dtype=f32)
    common = {
        "wqk": wqk, "wv1": wv1, "bqk": bqk, "bv1": bv1,
        "masks": masks, "ident": ident,
    }
    in_maps = []
    for b in range(B):
        m = dict(common)
        m["xT"] = np.ascontiguousarray(x[b].T, dtype=f32)  # [D, S]
        in_maps.append(m)
    return in_maps


def run(x, Wq, bq, Wk, bk, Wv, bv, trace=False):
    from concourse.bass_utils import run_bass_kernel_spmd

    if "nc" not in _CACHE:
        _CACHE["nc"] = _build_nc()
    nc = _CACHE["nc"]
    in_maps = _host_prep(
        np.asarray(x), np.asarray(Wq), np.asarray(bq), np.asarray(Wk),
        np.asarray(bk), np.asarray(Wv), np.asarray(bv),
    )
    res = run_bass_kernel_spmd(
        nc, in_maps, core_ids=list(range(N_CORES)), trace=trace
    )
    outs = np.stack([res.results[c]["out"] for c in range(N_CORES)], axis=0)
    return outs.astype(np.float32), res


def kernel(x, Wq, bq, Wk, bk, Wv, bv):
    outs, _ = run(x, Wq, bq, Wk, bk, Wv, bv, trace=False)
    return outs
